# revision 50
# baseline (speedup 1.0000x reference)
"""Trainium2 Bass kernel for nn_DAGLayer (gnn_message_passing).

Problem: out buffer holds L=256 leaf columns then M=512 computed nodes.
Node i gathers P=8 parent columns (each [N=32, C=256]) from the buffer,
applies y = einsum('ncp,ocp->no', g, W[i]) + b[i], and appends y.

Strategy (8 NeuronCores, one SPMD program):
  - Host computes a chain-aware schedule: nodes are packed into ~9 rounds
    of <=8 nodes per core; within a (round, core) sequence a node may
    consume outputs of earlier nodes in the same sequence (local chain),
    which cuts the number of global synchronizations from 21 levels to ~9.
  - The whole history (leaves + all node outputs) lives in SBUF as one
    fp16 tensor [128 part = c%128, slot*64 + (c//128)*32 + n].  Parent
    "gathers" are just dynamic-offset reads of that tensor by the PE:
    zero DMA traffic, zero descriptor pressure.
  - Weights are fp8e3 (E3M4), prescaled by 512 on the host so the values
    sit in the normal range (rel quant err ~1.4%, end-to-end ~8.5e-3).
    They stream HBM->SBUF at 0.5 MB/node (32 MB/core total) on the
    gpsimd SWDGE queues, 2 nodes per DMA, 8-buffer prefetch depth.
  - Per node: 32 accumulating matmuls, stationary = fp8e3 weight tile
    [128k x 128o] (fast-weight-load), moving = fp16 history slice
    [128, 32] at a register offset loaded from a per-core table.
    PSUM [128o, 32n] x2; scalar-engine activation applies 1/512 scale +
    per-partition bias and writes fp16 into a per-round staging tile.
  - Emission per round: phase A = matmuls whose parents are >=2 rounds
    old (overlap the in-flight AllGather), then the refresh DMA that
    copies the previous round's AllGather result into SBUF history, then
    phase B = fresh-parent matmuls, bias/activation, a DVE copy of each
    output into local history (for same-round chains), one staging->DRAM
    DMA, and the round's AllGather (skipped for the last round).

The kernel is self-contained; the schedule is derived from `parents` at
run time and the bass build is cached on the schedule signature.
"""

import hashlib
import os

import numpy as np

os.environ.setdefault("NEURON_COMPILE_CACHE_URL", "/root/neuron_cache")

NCORES = 8
QUOTA = 8
WSCALE = 512.0

_BUILD_CACHE = {}


def _schedule(parents, L, M):
    """Chain-aware greedy scheduler.

    Returns (s_list, node_of_coreslot[q, S], round_of_slot[S]).
    Node at core-slot s of round r may depend on: leaves, nodes of rounds
    < r (any core), or nodes at earlier slots of the same round on the
    same core.
    """
    children = [[] for _ in range(M)]
    for i in range(M):
        for p in parents[i]:
            if p >= L:
                children[p - L].append(i)
    height = np.zeros(M, np.int64)
    for i in range(M - 1, -1, -1):
        height[i] = 1 + max((height[c] for c in children[i]), default=0)
    order = sorted(range(M), key=lambda i: (-height[i], i))

    assigned = np.full(M, -1)
    core_of = np.full(M, -1)
    depth = np.zeros(M, np.int64)  # same-round chain depth of each node
    rounds = []
    remaining = order
    r = 0
    while remaining:
        cur = [[] for _ in range(NCORES)]
        done = set(i for i in range(M) if 0 <= assigned[i] < r)
        placed = set()
        for i in remaining:
            pending = [p - L for p in parents[i] if p >= L and (p - L) not in done]
            if not pending:
                q = min(range(NCORES), key=lambda q: len(cur[q]))
                if len(cur[q]) < QUOTA:
                    cur[q].append(i)
                    assigned[i] = r
                    core_of[i] = q
                    placed.add(i)
                    depth[i] = 1
            else:
                qs = set()
                ok = True
                d = 0
                for p in pending:
                    if assigned[p] == r and p in placed:
                        qs.add(core_of[p])
                        d = max(d, depth[p])
                    else:
                        ok = False
                        break
                # Chain depth is capped: every link in a same-round chain
                # adds ~1.5us to that core's serial tail, which both delays
                # the round's AllGather and skews it across cores (the mesh
                # waits for the slowest core).
                if ok and len(qs) == 1 and d < 3:
                    q = qs.pop()
                    if len(cur[q]) < QUOTA:
                        cur[q].append(i)
                        assigned[i] = r
                        core_of[i] = q
                        placed.add(i)
                        depth[i] = d + 1
        remaining = [i for i in remaining if i not in placed]
        rounds.append(cur)
        r += 1
        assert r <= 64, "scheduler failed to converge"

    # Within each (round, core) list, move nodes with only old parents
    # (leaf or rounds <= r-2) to the front: their copies/matmuls never wait
    # on the in-flight AllGather, so they fill the round-boundary window.
    # Stable partition preserves chain order (a chain child always has a
    # fresh same-round parent, so it stays behind it in the fresh group).
    for r, cur in enumerate(rounds):
        for q in range(NCORES):
            old = [
                i
                for i in cur[q]
                if all(p < L or assigned[p - L] < r - 1 for p in parents[i])
            ]
            fresh = [i for i in cur[q] if i not in old]
            cur[q] = old + fresh

    # pad round sizes to even (weight DMAs move node pairs)
    s_list = []
    for cur in rounds:
        s_r = max(len(c) for c in cur)
        s_r += s_r % 2
        s_list.append(s_r)
    S = sum(s_list)
    node_of_coreslot = np.full((NCORES, S), -1, np.int64)
    off = 0
    for cur, s_r in zip(rounds, s_list):
        for q in range(NCORES):
            for m, i in enumerate(cur[q]):
                node_of_coreslot[q, off + m] = i
        off += s_r
    return s_list, node_of_coreslot, assigned, core_of


def _plan_copy_engines(s_list, offs, old_counts):
    """Assign every (slot, tap-position) to a copy engine, balancing the
    measured per-copy engine costs.  Phase-A taps (old parents) use all
    three engines; phase-B taps (fresh parents) use DVE/Pool only, since
    the ACT queue holds the round's activations.  Returns eng_of[s][pos]
    with 0=DVE, 1=ACT, 2=Pool."""
    COST = [185.0, 300.0, 375.0]
    S = offs[-1]
    eng_of = np.zeros((S, 8), np.int64)
    for r in range(len(s_list)):
        load = [0.0, 0.0, 0.0]
        for m in range(s_list[r]):
            s = offs[r] + m
            for pos in range(old_counts[s]):
                e = min(range(3), key=lambda e: load[e] + COST[e])
                eng_of[s][pos] = e
                load[e] += COST[e]
        loadb = [0.0, 0.0]
        for m in range(s_list[r]):
            s = offs[r] + m
            for pos in range(old_counts[s], 8):
                e = min(range(2), key=lambda e: loadb[e] + COST[2 * e])
                eng_of[s][pos] = 2 * e  # DVE or Pool
                loadb[e] += COST[2 * e]
    return eng_of


def _build_bass(L, s_list, S, old_counts, needs_local, plan, vmin, vmax, lmin, lmax):
    """old_counts[s] = number of taps (after the host-side per-slot tap
    permutation putting old taps first) whose parent data is >= 2 rounds
    old on EVERY core; those matmuls are emitted before the previous
    round's refresh DMA so they never wait on the AllGather.

    plan: per round, a list of 3 entries (one per copy engine), each
    (gidx_start, [(m, pos), ...]) describing which staging positions that
    engine copies this round; the gidx input is laid out in exactly this
    order so each engine's offsets load with ONE register-load per round.
    vmin/vmax: per-gidx-position bounds (over cores) of the dynamic
    history gather offsets; lmin/lmax: [S] bounds of the local-history
    write offsets.  Tight bounds keep the tile framework's dependency
    ranges precise, so old-parent copies never falsely serialize against
    the round refresh or the local-history writes."""
    import concourse.bacc as bacc
    import concourse.bass as bass
    import concourse.mybir as mybir
    import concourse.tile as tile

    f16 = mybir.dt.float16
    f32 = mybir.dt.float32
    f8e3 = mybir.dt.float8e3
    i32 = mybir.dt.int32
    PE = mybir.EngineType.PE
    VEC = mybir.EngineType.DVE

    nslots = L + 8 * S  # total history slots
    HCOLS = nslots * 64  # fp16 elements per partition of the history

    nc = bacc.Bacc(num_devices=NCORES, num_swdge_queues=4)

    wbuf = nc.dram_tensor("wbuf", [S // 2, 128, 2, 16, 2, 128], f8e3, kind="ExternalInput")
    xt = nc.dram_tensor("xt", [128, L * 64], f16, kind="ExternalInput")
    bbuf = nc.dram_tensor("bbuf", [128, 2 * S], f32, kind="ExternalInput")
    gidx = nc.dram_tensor("gidx", [1, 8 * S], i32, kind="ExternalInput")
    lidx = nc.dram_tensor("lidx", [1, S], i32, kind="ExternalInput")
    yout = nc.dram_tensor("yout", [128, S * 64], f16, kind="ExternalOutput")
    rg = [list(range(NCORES))]

    nrounds = len(s_list)
    offs = np.concatenate([[0], np.cumsum(s_list)])
    agins = [
        nc.dram_tensor(f"agin{r}", [128, s_list[r] * 64], f16)
        for r in range(nrounds)
    ]
    hbufs = [
        nc.dram_tensor(
            f"hbuf{r}", [8 * 128, s_list[r] * 64], f16, addr_space="Shared"
        )
        for r in range(nrounds - 1)
    ]

    with tile.TileContext(nc) as tc:
        with (
            tc.tile_pool(name="const", bufs=1) as constp,
            tc.tile_pool(name="w", bufs=8) as wp,
            tc.tile_pool(name="stage", bufs=3) as stp,
            tc.tile_pool(name="g", bufs=12) as gp,
            tc.tile_pool(name="py", bufs=8, space="PSUM") as pyp,
        ):
            # tiny index tensors first so nothing blocks them
            gidx_sb = constp.tile([1, 8 * S], i32)
            nc.sync.dma_start(gidx_sb[:], gidx[:])
            lidx_sb = constp.tile([1, S], i32)
            nc.sync.dma_start(lidx_sb[:], lidx[:])

            hist = constp.tile([128, HCOLS], f16)
            # leaves, partition-major, 4 chunks over the two HWDGE rings
            qtr = (L // 4) * 64
            for k in range(4):
                (nc.sync if k % 2 == 0 else nc.scalar).dma_start(
                    hist[:, k * qtr : (k + 1) * qtr], xt[:, k * qtr : (k + 1) * qtr]
                )
            b_sb = constp.tile([128, 2 * S], f32)
            nc.sync.dma_start(b_sb[:], bbuf[:])

            ACT = mybir.EngineType.Activation
            POOL = mybir.EngineType.Pool
            cengines = [
                (nc.vector, VEC),
                (nc.scalar, ACT),
                (nc.gpsimd, POOL),
            ]

            def load_round_vals(r):
                """One register-load per copy engine covering all of its
                (slot, tap) offsets for round r, with tight per-value
                bounds.  Returns {(m, pos): ScalarValue}."""
                vals = {}
                CHUNK = 24  # TensorLoad supports at most 32 registers
                for e, (gstart, entries) in enumerate(plan[r]):
                    for c0 in range(0, len(entries), CHUNK):
                        part = entries[c0 : c0 + CHUNK]
                        g0 = gstart + c0
                        _, raw = nc.values_load_multi_w_load_instructions(
                            gidx_sb[0:1, g0 : g0 + len(part)],
                            engines=[cengines[e][1]],
                            min_val=int(min(vmin[g0 : g0 + len(part)])),
                            max_val=int(max(vmax[g0 : g0 + len(part)])),
                            skip_runtime_bounds_check=True,
                        )
                        for k, (m, pos) in enumerate(part):
                            vals[(m, pos)] = (
                                e,
                                nc.s_assert_within(
                                    raw[k],
                                    int(vmin[g0 + k]),
                                    int(vmax[g0 + k]),
                                    skip_runtime_assert=True,
                                ),
                            )
                return vals

            def emit_tap_copies(s, m, stg, lo, hi, vals):
                """Copy tap positions [lo, hi) of slot s from the (dynamic)
                history into the node's static staging tile, on the
                engines chosen by the plan."""
                for pos in range(lo, hi):
                    e, v = vals[(m, pos)]
                    eng, et = cengines[e]
                    dst = stg[:, pos, :]
                    srcv = hist[:, bass.ds(v, 64)]
                    if et == ACT:
                        nc.scalar.copy(dst, srcv)
                    else:
                        eng.tensor_copy(dst, srcv)

            def emit_node_mms(s, w_t, j, lo, hi, start, stop, pys, stg):
                """Static matmuls for tap positions [lo, hi) of slot s."""
                for i, t in enumerate(range(lo, hi)):
                    for h in range(2):
                        mv = stg[:, t, h * 32 : (h + 1) * 32]
                        for oh in range(2):
                            # start=True clears has_written for the WHOLE
                            # bank, so it must appear exactly once per node
                            # (the bank is per-node); later first-writes to
                            # untouched elements overwrite per-element.
                            nc.tensor.matmul(
                                pys[oh],
                                w_t[:, j, 2 * t + h, oh, :],
                                mv,
                                start=(start and i == 0 and h == 0 and oh == 0),
                                stop=(stop and t == hi - 1 and h == 1 and oh == 1),
                            )

            # Weight DMAs go on the HWDGE rings (sync/scalar), NOT the
            # gpsimd SWDGE queue: on gpsimd their pool-buffer WAW wait plus
            # ~1-2us/tile of Q7 descriptor generation blocked the Pool
            # engine's share of the next round's phase-A copies, which kept
            # the PE completely idle during every AllGather window.  They
            # are emitted two rounds ahead, after the round refresh, so
            # the refresh chunks are never queued behind 4 MB of weights.
            wts_of_round = {}

            def emit_weight_dmas(r):
                if r >= nrounds or r in wts_of_round:
                    return
                wts = []
                for pi in range(s_list[r] // 2):
                    w_t = wp.tile([128, 2, 16, 2, 128], f8e3, tag="w")
                    (nc.sync if pi % 2 == 0 else nc.scalar).dma_start(
                        w_t[:], wbuf[offs[r] // 2 + pi]
                    )
                    wts.append(w_t)
                wts_of_round[r] = wts

            emit_weight_dmas(0)
            emit_weight_dmas(1)

            pend_pys = {}
            stgs = {}
            round_vals = {}

            def emit_phase_a(r):
                """Phase A of round r: old-tap copies + matmuls.  Emitted
                right AFTER round r-1's phase B, with a scheduling-stage
                hint of r+0.7 (phase B of round r sits at r+1): the tile
                scheduler's internal sim models the AllGather as fast and
                would otherwise order round r's REFRESH-BLOCKED phase B
                ahead of this AG-independent work in the engine FIFOs,
                leaving every engine idle for the real ~13us AG."""
                off, s_r = offs[r], s_list[r]
                with tc.tile_wait_until(r + 0.7):
                    vals = load_round_vals(r)
                    round_vals[r] = vals
                    for m in range(s_r):
                        s = off + m
                        w_t, j = wts_of_round[r][m // 2], m % 2
                        n_old = old_counts[s]
                        py_t = pyp.tile([128, 64], f32, tag="py", name=f"py{s}")
                        pys = [py_t[:, 0:32], py_t[:, 32:64]]
                        pend_pys[s] = pys
                        stg = gp.tile([128, 8, 64], f16, tag="g")
                        stgs[s] = stg
                        emit_tap_copies(s, m, stg, 0, n_old, vals)
                        emit_node_mms(
                            s, w_t, j, 0, n_old,
                            start=True, stop=(n_old == 8), pys=pys, stg=stg,
                        )

            emit_phase_a(0)
            for r in range(nrounds):
                off = offs[r]
                s_r = s_list[r]
                wts = wts_of_round.pop(r)
                vals = round_vals.pop(r)
                ctx = tc.tile_wait_until(r + 1.0)
                ctx.__enter__()

                # refresh: previous round's AllGather result -> history,
                # one DMA per source core, alternating the HWDGE rings
                if r > 0:
                    poff, ps_r = offs[r - 1], s_list[r - 1]
                    gbase = L + 8 * poff
                    for q in range(NCORES):
                        dst = hist[
                            :,
                            (gbase + q * ps_r) * 64 : (gbase + (q + 1) * ps_r) * 64,
                        ]
                        src = hbufs[r - 1][q * 128 : (q + 1) * 128, :]
                        (nc.sync if q % 2 == 0 else nc.scalar).dma_start(dst, src)

                # weights for round r+2 queue right behind the refresh
                emit_weight_dmas(r + 2)

                # phase B: fresh-tap matmuls, bias, staging, local history
                stage = stp.tile([128, s_r * 64], f16, tag="stage")
                loc_lvs = {}
                if any(needs_local[off + m] for m in range(s_r)):
                    _, lraw = nc.values_load_multi_w_load_instructions(
                        lidx_sb[0:1, off : off + s_r],
                        engines=[VEC],
                        min_val=int(min(lmin[off : off + s_r])),
                        max_val=int(max(lmax[off : off + s_r])),
                        skip_runtime_bounds_check=True,
                    )
                    for m in range(s_r):
                        if needs_local[off + m]:
                            loc_lvs[m] = nc.s_assert_within(
                                lraw[m],
                                int(lmin[off + m]),
                                int(lmax[off + m]),
                                skip_runtime_assert=True,
                            )
                for m in range(s_r):
                    s = off + m
                    w_t, j = wts[m // 2], m % 2
                    n_old = old_counts[s]
                    pys = pend_pys.pop(s)
                    stg = stgs.pop(s)
                    # fresh copies go on DVE/Pool only (per the plan): the
                    # ACT queue holds the bias ops of earlier nodes, which
                    # would delay these copies (and so the PE) by a node.
                    emit_tap_copies(s, m, stg, n_old, 8, vals)
                    emit_node_mms(
                        s, w_t, j, n_old, 8,
                        start=(n_old == 0), stop=True, pys=pys, stg=stg,
                    )
                    for oh in range(2):
                        nc.scalar.activation(
                            stage[:, m * 64 + oh * 32 : m * 64 + (oh + 1) * 32],
                            pys[oh],
                            mybir.ActivationFunctionType.Identity,
                            bias=b_sb[:, 2 * s + oh : 2 * s + oh + 1],
                            scale=1.0 / WSCALE,
                        )
                    if needs_local[s]:
                        nc.vector.tensor_copy(
                            hist[:, bass.ds(loc_lvs[m], 64)],
                            stage[:, m * 64 : (m + 1) * 64],
                        )

                # staging -> DRAM round blocks (AG input + host output).
                # The AG input is copied in two halves so the first half
                # overlaps the last slots' activations and the AG trigger
                # fires ~a transfer earlier.
                if r < nrounds - 1:
                    hcols = (s_r // 2) * 64
                    nc.sync.dma_start(agins[r][:, :hcols], stage[:, :hcols])
                    nc.sync.dma_start(agins[r][:, hcols:], stage[:, hcols:])
                nc.scalar.dma_start(
                    yout[:, offs[r] * 64 : (offs[r] + s_r) * 64], stage[:]
                )

                ctx.__exit__(None, None, None)
                if r < nrounds - 1:
                    # next round's AG-independent work is EMITTED (and
                    # stage-hinted) before the AG so the Pool queue runs
                    # its copies before blocking at the AG trigger; the
                    # trigger then fires exactly when agin lands.
                    emit_phase_a(r + 1)
                    with tc.tile_wait_until(r + 1.75):
                        nc.gpsimd.collective_compute(
                            "AllGather",
                            mybir.AluOpType.bypass,
                            replica_groups=rg,
                            ins=[agins[r][:]],
                            outs=[hbufs[r][:]],
                        )
    nc.compile()
    return nc


def kernel(x, W, b, parents):
    import ml_dtypes
    from concourse.bass_utils import run_bass_kernel_spmd

    x = np.ascontiguousarray(np.asarray(x), dtype=np.float32)
    W = np.ascontiguousarray(np.asarray(W), dtype=np.float32)
    b = np.ascontiguousarray(np.asarray(b), dtype=np.float32)
    parents = np.asarray(parents).astype(np.int64)

    N, C, L = x.shape
    M, O, C2, P = W.shape
    assert (N, C, O, C2, P) == (32, 256, 256, 256, 8), "kernel hardcodes these dims"

    s_list, node_of_coreslot, round_of, core_of = _schedule(parents, L, M)
    S = sum(s_list)
    nrounds = len(s_list)
    offs = np.concatenate([[0], np.cumsum(s_list)])
    round_of_slot = np.zeros(S, np.int64)
    for r in range(nrounds):
        round_of_slot[offs[r] : offs[r + 1]] = r

    # global history slot of each node: leaves 0..L-1, then computed slots
    # rank-major per round (AllGather concatenation order).
    gslot = np.full(L + M, -1, np.int64)
    gslot[:L] = np.arange(L)
    slot_in_core = np.full(M, -1, np.int64)  # core-slot index s of node
    for q in range(NCORES):
        for s in range(S):
            i = node_of_coreslot[q, s]
            if i >= 0:
                r = round_of_slot[s]
                m = s - offs[r]
                gslot[L + i] = L + 8 * offs[r] + q * s_list[r] + m
                slot_in_core[i] = s
    assert (gslot[L:] >= 0).all()

    # Tap ordering is per-core DATA (weights + gidx follow the same
    # permutation), so each core puts its own "old" taps (parent is a
    # leaf or >= 2 rounds old) first.  The static program only needs the
    # per-slot phase-A count = min over cores of the old-tap count.
    perm_qs = np.tile(np.arange(P), (NCORES, S, 1))
    old_counts = [P] * S
    needs_local = [False] * S
    for s in range(S):
        r = round_of_slot[s]
        nmin = P
        for q in range(NCORES):
            i = node_of_coreslot[q, s]
            if i < 0:
                continue
            old, fresh = [], []
            for tap in range(P):
                par = parents[i][tap]
                if par >= L and round_of[par - L] >= r - 1:
                    fresh.append(tap)
                    if round_of[par - L] == r:
                        needs_local[slot_in_core[par - L]] = True
                else:
                    old.append(tap)
            perm_qs[q, s] = old + fresh
            nmin = min(nmin, len(old))
        old_counts[s] = nmin

    # weight relayout: [M, o, c, p] -> [128 part=c%128, ktile=(tap', c//128),
    # oh, o%128] with taps permuted so old taps come first; pairs of slots
    # per DMA tile.
    Wp = W * WSCALE
    in_maps = []
    gq_all = np.zeros((NCORES, 1, 8 * S), np.int32)
    lq_all = np.zeros((NCORES, 1, S), np.int32)
    xt_host = np.ascontiguousarray(
        x.transpose(2, 1, 0)
        .reshape(L, 2, 128, 32)
        .transpose(2, 0, 1, 3)
        .reshape(128, L * 64)
        .astype(np.float16)
    )
    # copy-engine plan + engine-grouped gidx layout: per round, each copy
    # engine's offsets occupy one contiguous gidx range (phase-A entries
    # first) so they load with a single register-load per round.
    eng_of = _plan_copy_engines(s_list, offs, old_counts)
    plan = []
    gpos_of = np.zeros((S, 8), np.int64)
    gpos = 0
    for r in range(nrounds):
        rplan = []
        for e in range(3):
            entries = []
            for phase in range(2):
                for m in range(s_list[r]):
                    s = offs[r] + m
                    lo, hi = (0, old_counts[s]) if phase == 0 else (old_counts[s], 8)
                    for pos in range(lo, hi):
                        if eng_of[s][pos] == e:
                            entries.append((m, pos))
            rplan.append((gpos, entries))
            for k, (m, pos) in enumerate(entries):
                gpos_of[offs[r] + m][pos] = gpos + k
            gpos += len(entries)
        plan.append(rplan)
    assert gpos == 8 * S

    for q in range(NCORES):
        Wq = np.zeros((S, 128, 16, 2, 128), np.float32)
        bq = np.zeros((S, 2, 128), np.float32)
        for s in range(S):
            i = node_of_coreslot[q, s]
            lq_all[q, 0, s] = gslot[L + i] * 64 if i >= 0 else (L + 8 * offs[round_of_slot[s]] + q * s_list[round_of_slot[s]] + (s - offs[round_of_slot[s]])) * 64
            if i < 0:
                continue
            # W[i]: [o, c, p] -> permuted taps -> [p', c, o] -> k-major
            wi = Wp[i][:, :, perm_qs[q, s]]  # [o, c, p']
            wi = wi.transpose(2, 1, 0).reshape(16, 128, 2, 128).transpose(1, 0, 2, 3)
            Wq[s] = wi
            bq[s] = b[i].reshape(2, 128)
            pslots = gslot[parents[i][perm_qs[q, s]]]
            for pos in range(P):
                gq_all[q, 0, gpos_of[s][pos]] = pslots[pos] * 64
        Wq8 = (
            Wq.reshape(S // 2, 2, 128, 16, 2, 128)
            .transpose(0, 2, 1, 3, 4, 5)
            .astype(ml_dtypes.float8_e3m4)
        )
        bq2 = np.ascontiguousarray(bq.transpose(2, 0, 1).reshape(128, 2 * S))
        in_maps.append(
            {
                "wbuf": np.ascontiguousarray(Wq8),
                "xt": xt_host,
                "bbuf": bq2,
                "gidx": gq_all[q],
                "lidx": lq_all[q],
            }
        )

    vmin = gq_all.min(axis=0)[0]
    vmax = gq_all.max(axis=0)[0]
    lmin = lq_all.min(axis=0)[0]
    lmax = lq_all.max(axis=0)[0]

    sig = hashlib.sha1(
        vmin.tobytes()
        + vmax.tobytes()
        + lmin.tobytes()
        + lmax.tobytes()
        + np.asarray(old_counts).tobytes()
        + np.asarray(needs_local).tobytes()
        + eng_of.tobytes()
    ).hexdigest()
    key = (L, tuple(s_list), sig)
    if key not in _BUILD_CACHE:
        import time as _time

        _t0 = _time.time()
        _BUILD_CACHE[key] = _build_bass(
            L, s_list, S, old_counts, needs_local, plan, vmin, vmax, lmin, lmax
        )
        print(f"[kernel] bass build took {_time.time() - _t0:.1f}s", flush=True)
    nc = _BUILD_CACHE[key]

    global LAST_RUN
    LAST_RUN = (nc, in_maps)

    results = run_bass_kernel_spmd(nc, in_maps, core_ids=list(range(NCORES))).results

    out = np.zeros((N, C, L + M), np.float32)
    out[:, :, :L] = x
    for q in range(NCORES):
        yq = (
            np.asarray(results[q]["yout"])
            .astype(np.float32)
            .reshape(128, S, 2, 32)
            .transpose(1, 3, 2, 0)
            .reshape(S, 32, 256)
        )
        for s in range(S):
            i = node_of_coreslot[q, s]
            if i >= 0:
                out[:, :, L + i] = yq[s]
    return out



# revision 52
# speedup vs baseline: 1.1171x; 1.1171x over previous
"""Trainium2 Bass kernel for nn_DAGLayer (gnn_message_passing).

Problem: out buffer holds L=256 leaf columns then M=512 computed nodes.
Node i gathers P=8 parent columns (each [N=32, C=256]) from the buffer,
applies y = einsum('ncp,ocp->no', g, W[i]) + b[i], and appends y.

Strategy (8 NeuronCores, one SPMD program):
  - Host computes a chain-aware schedule: nodes are packed into ~9 rounds
    of <=8 nodes per core; within a (round, core) sequence a node may
    consume outputs of earlier nodes in the same sequence (local chain),
    which cuts the number of global synchronizations from 21 levels to ~9.
  - The whole history (leaves + all node outputs) lives in SBUF as one
    fp16 tensor [128 part = c%128, slot*64 + (c//128)*32 + n].  Parent
    "gathers" are just dynamic-offset reads of that tensor by the PE:
    zero DMA traffic, zero descriptor pressure.
  - Weights are fp8e3 (E3M4), prescaled by 512 on the host so the values
    sit in the normal range (rel quant err ~1.4%, end-to-end ~8.5e-3).
    They stream HBM->SBUF at 0.5 MB/node (32 MB/core total) on the
    gpsimd SWDGE queues, 2 nodes per DMA, 8-buffer prefetch depth.
  - Per node: 32 accumulating matmuls, stationary = fp8e3 weight tile
    [128k x 128o] (fast-weight-load), moving = fp16 history slice
    [128, 32] at a register offset loaded from a per-core table.
    PSUM [128o, 32n] x2; scalar-engine activation applies 1/512 scale +
    per-partition bias and writes fp16 into a per-round staging tile.
  - Emission per round: phase A = matmuls whose parents are >=2 rounds
    old (overlap the in-flight AllGather), then the refresh DMA that
    copies the previous round's AllGather result into SBUF history, then
    phase B = fresh-parent matmuls, bias/activation, a DVE copy of each
    output into local history (for same-round chains), one staging->DRAM
    DMA, and the round's AllGather (skipped for the last round).

The kernel is self-contained; the schedule is derived from `parents` at
run time and the bass build is cached on the schedule signature.
"""

import hashlib
import os

import numpy as np

os.environ.setdefault("NEURON_COMPILE_CACHE_URL", "/root/neuron_cache")

NCORES = 8
QUOTA = 8
WSCALE = 512.0

_BUILD_CACHE = {}


def _schedule(parents, L, M):
    """Chain-aware greedy scheduler.

    Returns (s_list, node_of_coreslot[q, S], round_of_slot[S]).
    Node at core-slot s of round r may depend on: leaves, nodes of rounds
    < r (any core), or nodes at earlier slots of the same round on the
    same core.
    """
    children = [[] for _ in range(M)]
    for i in range(M):
        for p in parents[i]:
            if p >= L:
                children[p - L].append(i)
    height = np.zeros(M, np.int64)
    for i in range(M - 1, -1, -1):
        height[i] = 1 + max((height[c] for c in children[i]), default=0)
    order = sorted(range(M), key=lambda i: (-height[i], i))

    assigned = np.full(M, -1)
    core_of = np.full(M, -1)
    rounds = []
    remaining = order
    r = 0
    while remaining:
        cur = [[] for _ in range(NCORES)]
        done = set(i for i in range(M) if 0 <= assigned[i] < r)
        placed = set()
        for i in remaining:
            pending = [p - L for p in parents[i] if p >= L and (p - L) not in done]
            if not pending:
                q = min(range(NCORES), key=lambda q: len(cur[q]))
                if len(cur[q]) < QUOTA:
                    cur[q].append(i)
                    assigned[i] = r
                    core_of[i] = q
                    placed.add(i)
            else:
                qs = set()
                ok = True
                for p in pending:
                    if assigned[p] == r and p in placed:
                        qs.add(core_of[p])
                    else:
                        ok = False
                        break
                if ok and len(qs) == 1:
                    q = qs.pop()
                    if len(cur[q]) < QUOTA:
                        cur[q].append(i)
                        assigned[i] = r
                        core_of[i] = q
                        placed.add(i)
        remaining = [i for i in remaining if i not in placed]
        rounds.append(cur)
        r += 1
        assert r <= 64, "scheduler failed to converge"

    # Within each (round, core) list, move nodes with only old parents
    # (leaf or rounds <= r-2) to the front: their copies/matmuls never wait
    # on the in-flight AllGather, so they fill the round-boundary window.
    # Stable partition preserves chain order (a chain child always has a
    # fresh same-round parent, so it stays behind it in the fresh group).
    for r, cur in enumerate(rounds):
        for q in range(NCORES):
            old = [
                i
                for i in cur[q]
                if all(p < L or assigned[p - L] < r - 1 for p in parents[i])
            ]
            fresh = [i for i in cur[q] if i not in old]
            cur[q] = old + fresh

    # pad round sizes to even (weight DMAs move node pairs)
    s_list = []
    for cur in rounds:
        s_r = max(len(c) for c in cur)
        s_r += s_r % 2
        s_list.append(s_r)
    S = sum(s_list)
    node_of_coreslot = np.full((NCORES, S), -1, np.int64)
    off = 0
    for cur, s_r in zip(rounds, s_list):
        for q in range(NCORES):
            for m, i in enumerate(cur[q]):
                node_of_coreslot[q, off + m] = i
        off += s_r
    return s_list, node_of_coreslot, assigned, core_of


def _plan_copy_engines(s_list, offs, old_counts):
    """Assign every (slot, tap-position) to a copy engine, balancing the
    measured per-copy engine costs.  Phase-A taps (old parents) use all
    three engines; phase-B taps (fresh parents) use DVE/Pool only, since
    the ACT queue holds the round's activations.  Returns eng_of[s][pos]
    with 0=DVE, 1=ACT, 2=Pool."""
    COST = [185.0, 300.0, 375.0]
    S = offs[-1]
    eng_of = np.zeros((S, 8), np.int64)
    for r in range(len(s_list)):
        load = [0.0, 0.0, 0.0]
        for m in range(s_list[r]):
            s = offs[r] + m
            for pos in range(old_counts[s]):
                e = min(range(3), key=lambda e: load[e] + COST[e])
                eng_of[s][pos] = e
                load[e] += COST[e]
        loadb = [0.0, 0.0]
        for m in range(s_list[r]):
            s = offs[r] + m
            for pos in range(old_counts[s], 8):
                e = min(range(2), key=lambda e: loadb[e] + COST[2 * e])
                eng_of[s][pos] = 2 * e  # DVE or Pool
                loadb[e] += COST[2 * e]
    return eng_of


def _build_bass(L, s_list, S, old_counts, needs_local, plan, vmin, vmax, lmin, lmax):
    """old_counts[s] = number of taps (after the host-side per-slot tap
    permutation putting old taps first) whose parent data is >= 2 rounds
    old on EVERY core; those matmuls are emitted before the previous
    round's refresh DMA so they never wait on the AllGather.

    plan: per round, a list of 3 entries (one per copy engine), each
    (gidx_start, [(m, pos), ...]) describing which staging positions that
    engine copies this round; the gidx input is laid out in exactly this
    order so each engine's offsets load with ONE register-load per round.
    vmin/vmax: per-gidx-position bounds (over cores) of the dynamic
    history gather offsets; lmin/lmax: [S] bounds of the local-history
    write offsets.  Tight bounds keep the tile framework's dependency
    ranges precise, so old-parent copies never falsely serialize against
    the round refresh or the local-history writes."""
    import concourse.bacc as bacc
    import concourse.bass as bass
    import concourse.mybir as mybir
    import concourse.tile as tile

    f16 = mybir.dt.float16
    f32 = mybir.dt.float32
    f8e3 = mybir.dt.float8e3
    i32 = mybir.dt.int32
    PE = mybir.EngineType.PE
    VEC = mybir.EngineType.DVE

    nslots = L + 8 * S  # total history slots
    HCOLS = nslots * 64  # fp16 elements per partition of the history

    nc = bacc.Bacc(num_devices=NCORES, num_swdge_queues=4)

    wbuf = nc.dram_tensor("wbuf", [S // 2, 128, 2, 16, 2, 128], f8e3, kind="ExternalInput")
    xt = nc.dram_tensor("xt", [128, L * 64], f16, kind="ExternalInput")
    bbuf = nc.dram_tensor("bbuf", [128, 2 * S], f32, kind="ExternalInput")
    gidx = nc.dram_tensor("gidx", [1, 8 * S], i32, kind="ExternalInput")
    lidx = nc.dram_tensor("lidx", [1, S], i32, kind="ExternalInput")
    yout = nc.dram_tensor("yout", [128, S * 64], f16, kind="ExternalOutput")
    rg = [list(range(NCORES))]

    nrounds = len(s_list)
    offs = np.concatenate([[0], np.cumsum(s_list)])
    agins = [
        nc.dram_tensor(f"agin{r}", [128, s_list[r] * 64], f16)
        for r in range(nrounds)
    ]
    hbufs = [
        nc.dram_tensor(
            f"hbuf{r}", [8 * 128, s_list[r] * 64], f16, addr_space="Shared"
        )
        for r in range(nrounds - 1)
    ]

    with tile.TileContext(nc) as tc:
        with (
            tc.tile_pool(name="const", bufs=1) as constp,
            tc.tile_pool(name="w", bufs=8) as wp,
            tc.tile_pool(name="stage", bufs=3) as stp,
            tc.tile_pool(name="g", bufs=12) as gp,
            tc.tile_pool(name="py", bufs=8, space="PSUM") as pyp,
        ):
            # tiny index tensors first so nothing blocks them
            gidx_sb = constp.tile([1, 8 * S], i32)
            nc.sync.dma_start(gidx_sb[:], gidx[:])
            lidx_sb = constp.tile([1, S], i32)
            nc.sync.dma_start(lidx_sb[:], lidx[:])

            hist = constp.tile([128, HCOLS], f16)
            # leaves, partition-major, 4 chunks over the two HWDGE rings
            qtr = (L // 4) * 64
            for k in range(4):
                (nc.sync if k % 2 == 0 else nc.scalar).dma_start(
                    hist[:, k * qtr : (k + 1) * qtr], xt[:, k * qtr : (k + 1) * qtr]
                )
            b_sb = constp.tile([128, 2 * S], f32)
            nc.sync.dma_start(b_sb[:], bbuf[:])

            ACT = mybir.EngineType.Activation
            POOL = mybir.EngineType.Pool
            cengines = [
                (nc.vector, VEC),
                (nc.scalar, ACT),
                (nc.gpsimd, POOL),
            ]

            def load_round_vals(r):
                """One register-load per copy engine covering all of its
                (slot, tap) offsets for round r, with tight per-value
                bounds.  Returns {(m, pos): ScalarValue}."""
                vals = {}
                CHUNK = 24  # TensorLoad supports at most 32 registers
                for e, (gstart, entries) in enumerate(plan[r]):
                    for c0 in range(0, len(entries), CHUNK):
                        part = entries[c0 : c0 + CHUNK]
                        g0 = gstart + c0
                        _, raw = nc.values_load_multi_w_load_instructions(
                            gidx_sb[0:1, g0 : g0 + len(part)],
                            engines=[cengines[e][1]],
                            min_val=int(min(vmin[g0 : g0 + len(part)])),
                            max_val=int(max(vmax[g0 : g0 + len(part)])),
                            skip_runtime_bounds_check=True,
                        )
                        for k, (m, pos) in enumerate(part):
                            vals[(m, pos)] = (
                                e,
                                nc.s_assert_within(
                                    raw[k],
                                    int(vmin[g0 + k]),
                                    int(vmax[g0 + k]),
                                    skip_runtime_assert=True,
                                ),
                            )
                return vals

            def emit_tap_copies(s, m, stg, lo, hi, vals):
                """Copy tap positions [lo, hi) of slot s from the (dynamic)
                history into the node's static staging tile, on the
                engines chosen by the plan."""
                for pos in range(lo, hi):
                    e, v = vals[(m, pos)]
                    eng, et = cengines[e]
                    dst = stg[:, pos, :]
                    srcv = hist[:, bass.ds(v, 64)]
                    if et == ACT:
                        nc.scalar.copy(dst, srcv)
                    else:
                        eng.tensor_copy(dst, srcv)

            def emit_node_mms(s, w_t, j, lo, hi, start, stop, pys, stg):
                """Static matmuls for tap positions [lo, hi) of slot s."""
                for i, t in enumerate(range(lo, hi)):
                    for h in range(2):
                        mv = stg[:, t, h * 32 : (h + 1) * 32]
                        for oh in range(2):
                            # start=True clears has_written for the WHOLE
                            # bank, so it must appear exactly once per node
                            # (the bank is per-node); later first-writes to
                            # untouched elements overwrite per-element.
                            nc.tensor.matmul(
                                pys[oh],
                                w_t[:, j, 2 * t + h, oh, :],
                                mv,
                                start=(start and i == 0 and h == 0 and oh == 0),
                                stop=(stop and t == hi - 1 and h == 1 and oh == 1),
                            )

            # Weight DMAs go on the HWDGE rings (sync/scalar), NOT the
            # gpsimd SWDGE queue: on gpsimd their pool-buffer WAW wait plus
            # ~1-2us/tile of Q7 descriptor generation blocked the Pool
            # engine's share of the next round's phase-A copies, which kept
            # the PE completely idle during every AllGather window.  They
            # are emitted two rounds ahead, after the round refresh, so
            # the refresh chunks are never queued behind 4 MB of weights.
            wts_of_round = {}

            def emit_weight_dmas(r):
                if r >= nrounds or r in wts_of_round:
                    return
                wts = []
                for pi in range(s_list[r] // 2):
                    w_t = wp.tile([128, 2, 16, 2, 128], f8e3, tag="w")
                    (nc.sync if pi % 2 == 0 else nc.scalar).dma_start(
                        w_t[:], wbuf[offs[r] // 2 + pi]
                    )
                    wts.append(w_t)
                wts_of_round[r] = wts

            emit_weight_dmas(0)
            emit_weight_dmas(1)

            pend_pys = {}
            stgs = {}
            round_vals = {}

            def emit_phase_a(r):
                """Phase A of round r: old-tap copies + matmuls.  Emitted
                right AFTER round r-1's phase B, with a scheduling-stage
                hint of r+0.7 (phase B of round r sits at r+1): the tile
                scheduler's internal sim models the AllGather as fast and
                would otherwise order round r's REFRESH-BLOCKED phase B
                ahead of this AG-independent work in the engine FIFOs,
                leaving every engine idle for the real ~13us AG."""
                off, s_r = offs[r], s_list[r]
                with tc.tile_wait_until(r + 0.7):
                    vals = load_round_vals(r)
                    round_vals[r] = vals
                    for m in range(s_r):
                        s = off + m
                        w_t, j = wts_of_round[r][m // 2], m % 2
                        n_old = old_counts[s]
                        py_t = pyp.tile([128, 64], f32, tag="py", name=f"py{s}")
                        pys = [py_t[:, 0:32], py_t[:, 32:64]]
                        pend_pys[s] = pys
                        stg = gp.tile([128, 8, 64], f16, tag="g")
                        stgs[s] = stg
                        emit_tap_copies(s, m, stg, 0, n_old, vals)
                        emit_node_mms(
                            s, w_t, j, 0, n_old,
                            start=True, stop=(n_old == 8), pys=pys, stg=stg,
                        )

            emit_phase_a(0)
            for r in range(nrounds):
                off = offs[r]
                s_r = s_list[r]
                wts = wts_of_round.pop(r)
                vals = round_vals.pop(r)
                ctx = tc.tile_wait_until(r + 1.0)
                ctx.__enter__()

                # refresh: previous round's AllGather result -> history,
                # one DMA per source core, alternating the HWDGE rings
                if r > 0:
                    poff, ps_r = offs[r - 1], s_list[r - 1]
                    gbase = L + 8 * poff
                    for q in range(NCORES):
                        dst = hist[
                            :,
                            (gbase + q * ps_r) * 64 : (gbase + (q + 1) * ps_r) * 64,
                        ]
                        src = hbufs[r - 1][q * 128 : (q + 1) * 128, :]
                        (nc.sync if q % 2 == 0 else nc.scalar).dma_start(dst, src)

                # weights for round r+2 queue right behind the refresh
                emit_weight_dmas(r + 2)

                # phase B: fresh-tap matmuls, bias, staging, local history
                stage = stp.tile([128, s_r * 64], f16, tag="stage")
                loc_lvs = {}
                if any(needs_local[off + m] for m in range(s_r)):
                    _, lraw = nc.values_load_multi_w_load_instructions(
                        lidx_sb[0:1, off : off + s_r],
                        engines=[VEC],
                        min_val=int(min(lmin[off : off + s_r])),
                        max_val=int(max(lmax[off : off + s_r])),
                        skip_runtime_bounds_check=True,
                    )
                    for m in range(s_r):
                        if needs_local[off + m]:
                            loc_lvs[m] = nc.s_assert_within(
                                lraw[m],
                                int(lmin[off + m]),
                                int(lmax[off + m]),
                                skip_runtime_assert=True,
                            )
                for m in range(s_r):
                    s = off + m
                    w_t, j = wts[m // 2], m % 2
                    n_old = old_counts[s]
                    pys = pend_pys.pop(s)
                    stg = stgs.pop(s)
                    # fresh copies go on DVE/Pool only (per the plan): the
                    # ACT queue holds the bias ops of earlier nodes, which
                    # would delay these copies (and so the PE) by a node.
                    emit_tap_copies(s, m, stg, n_old, 8, vals)
                    emit_node_mms(
                        s, w_t, j, n_old, 8,
                        start=(n_old == 0), stop=True, pys=pys, stg=stg,
                    )
                    for oh in range(2):
                        nc.scalar.activation(
                            stage[:, m * 64 + oh * 32 : m * 64 + (oh + 1) * 32],
                            pys[oh],
                            mybir.ActivationFunctionType.Identity,
                            bias=b_sb[:, 2 * s + oh : 2 * s + oh + 1],
                            scale=1.0 / WSCALE,
                        )
                    if needs_local[s]:
                        nc.vector.tensor_copy(
                            hist[:, bass.ds(loc_lvs[m], 64)],
                            stage[:, m * 64 : (m + 1) * 64],
                        )

                # staging -> DRAM round blocks (AG input + host output).
                # The AG input is copied in four chunks over both HWDGE
                # rings: each fires as soon as its slots' activations land,
                # so only the last ~32KB chunk separates the final act from
                # the AG trigger.
                if r < nrounds - 1:
                    nchunk = min(4, s_r)
                    bnds = [(s_r * k) // nchunk * 64 for k in range(nchunk + 1)]
                    for k in range(nchunk):
                        (nc.sync if k % 2 == 0 else nc.scalar).dma_start(
                            agins[r][:, bnds[k] : bnds[k + 1]],
                            stage[:, bnds[k] : bnds[k + 1]],
                        )
                nc.scalar.dma_start(
                    yout[:, offs[r] * 64 : (offs[r] + s_r) * 64], stage[:]
                )

                ctx.__exit__(None, None, None)
                if r < nrounds - 1:
                    # next round's AG-independent work is EMITTED (and
                    # stage-hinted) before the AG so the Pool queue runs
                    # its copies before blocking at the AG trigger; the
                    # trigger then fires exactly when agin lands.
                    emit_phase_a(r + 1)
                    with tc.tile_wait_until(r + 1.75):
                        nc.gpsimd.collective_compute(
                            "AllGather",
                            mybir.AluOpType.bypass,
                            replica_groups=rg,
                            ins=[agins[r][:]],
                            outs=[hbufs[r][:]],
                        )
    nc.compile()
    return nc


def kernel(x, W, b, parents):
    import ml_dtypes
    from concourse.bass_utils import run_bass_kernel_spmd

    x = np.ascontiguousarray(np.asarray(x), dtype=np.float32)
    W = np.ascontiguousarray(np.asarray(W), dtype=np.float32)
    b = np.ascontiguousarray(np.asarray(b), dtype=np.float32)
    parents = np.asarray(parents).astype(np.int64)

    N, C, L = x.shape
    M, O, C2, P = W.shape
    assert (N, C, O, C2, P) == (32, 256, 256, 256, 8), "kernel hardcodes these dims"

    s_list, node_of_coreslot, round_of, core_of = _schedule(parents, L, M)
    S = sum(s_list)
    nrounds = len(s_list)
    offs = np.concatenate([[0], np.cumsum(s_list)])
    round_of_slot = np.zeros(S, np.int64)
    for r in range(nrounds):
        round_of_slot[offs[r] : offs[r + 1]] = r

    # global history slot of each node: leaves 0..L-1, then computed slots
    # rank-major per round (AllGather concatenation order).
    gslot = np.full(L + M, -1, np.int64)
    gslot[:L] = np.arange(L)
    slot_in_core = np.full(M, -1, np.int64)  # core-slot index s of node
    for q in range(NCORES):
        for s in range(S):
            i = node_of_coreslot[q, s]
            if i >= 0:
                r = round_of_slot[s]
                m = s - offs[r]
                gslot[L + i] = L + 8 * offs[r] + q * s_list[r] + m
                slot_in_core[i] = s
    assert (gslot[L:] >= 0).all()

    # Tap ordering is per-core DATA (weights + gidx follow the same
    # permutation), so each core puts its own "old" taps (parent is a
    # leaf or >= 2 rounds old) first.  The static program only needs the
    # per-slot phase-A count = min over cores of the old-tap count.
    perm_qs = np.tile(np.arange(P), (NCORES, S, 1))
    old_counts = [P] * S
    needs_local = [False] * S
    for s in range(S):
        r = round_of_slot[s]
        nmin = P
        for q in range(NCORES):
            i = node_of_coreslot[q, s]
            if i < 0:
                continue
            old, fresh = [], []
            for tap in range(P):
                par = parents[i][tap]
                if par >= L and round_of[par - L] >= r - 1:
                    fresh.append(tap)
                    if round_of[par - L] == r:
                        needs_local[slot_in_core[par - L]] = True
                else:
                    old.append(tap)
            perm_qs[q, s] = old + fresh
            nmin = min(nmin, len(old))
        old_counts[s] = nmin

    # weight relayout: [M, o, c, p] -> [128 part=c%128, ktile=(tap', c//128),
    # oh, o%128] with taps permuted so old taps come first; pairs of slots
    # per DMA tile.
    Wp = W * WSCALE
    in_maps = []
    gq_all = np.zeros((NCORES, 1, 8 * S), np.int32)
    lq_all = np.zeros((NCORES, 1, S), np.int32)
    xt_host = np.ascontiguousarray(
        x.transpose(2, 1, 0)
        .reshape(L, 2, 128, 32)
        .transpose(2, 0, 1, 3)
        .reshape(128, L * 64)
        .astype(np.float16)
    )
    # copy-engine plan + engine-grouped gidx layout: per round, each copy
    # engine's offsets occupy one contiguous gidx range (phase-A entries
    # first) so they load with a single register-load per round.
    eng_of = _plan_copy_engines(s_list, offs, old_counts)
    plan = []
    gpos_of = np.zeros((S, 8), np.int64)
    gpos = 0
    for r in range(nrounds):
        rplan = []
        for e in range(3):
            entries = []
            for phase in range(2):
                for m in range(s_list[r]):
                    s = offs[r] + m
                    lo, hi = (0, old_counts[s]) if phase == 0 else (old_counts[s], 8)
                    for pos in range(lo, hi):
                        if eng_of[s][pos] == e:
                            entries.append((m, pos))
            rplan.append((gpos, entries))
            for k, (m, pos) in enumerate(entries):
                gpos_of[offs[r] + m][pos] = gpos + k
            gpos += len(entries)
        plan.append(rplan)
    assert gpos == 8 * S

    for q in range(NCORES):
        Wq = np.zeros((S, 128, 16, 2, 128), np.float32)
        bq = np.zeros((S, 2, 128), np.float32)
        for s in range(S):
            i = node_of_coreslot[q, s]
            lq_all[q, 0, s] = gslot[L + i] * 64 if i >= 0 else (L + 8 * offs[round_of_slot[s]] + q * s_list[round_of_slot[s]] + (s - offs[round_of_slot[s]])) * 64
            if i < 0:
                continue
            # W[i]: [o, c, p] -> permuted taps -> [p', c, o] -> k-major
            wi = Wp[i][:, :, perm_qs[q, s]]  # [o, c, p']
            wi = wi.transpose(2, 1, 0).reshape(16, 128, 2, 128).transpose(1, 0, 2, 3)
            Wq[s] = wi
            bq[s] = b[i].reshape(2, 128)
            pslots = gslot[parents[i][perm_qs[q, s]]]
            for pos in range(P):
                gq_all[q, 0, gpos_of[s][pos]] = pslots[pos] * 64
        Wq8 = (
            Wq.reshape(S // 2, 2, 128, 16, 2, 128)
            .transpose(0, 2, 1, 3, 4, 5)
            .astype(ml_dtypes.float8_e3m4)
        )
        bq2 = np.ascontiguousarray(bq.transpose(2, 0, 1).reshape(128, 2 * S))
        in_maps.append(
            {
                "wbuf": np.ascontiguousarray(Wq8),
                "xt": xt_host,
                "bbuf": bq2,
                "gidx": gq_all[q],
                "lidx": lq_all[q],
            }
        )

    vmin = gq_all.min(axis=0)[0]
    vmax = gq_all.max(axis=0)[0]
    lmin = lq_all.min(axis=0)[0]
    lmax = lq_all.max(axis=0)[0]

    sig = hashlib.sha1(
        vmin.tobytes()
        + vmax.tobytes()
        + lmin.tobytes()
        + lmax.tobytes()
        + np.asarray(old_counts).tobytes()
        + np.asarray(needs_local).tobytes()
        + eng_of.tobytes()
    ).hexdigest()
    key = (L, tuple(s_list), sig)
    if key not in _BUILD_CACHE:
        import time as _time

        _t0 = _time.time()
        _BUILD_CACHE[key] = _build_bass(
            L, s_list, S, old_counts, needs_local, plan, vmin, vmax, lmin, lmax
        )
        print(f"[kernel] bass build took {_time.time() - _t0:.1f}s", flush=True)
    nc = _BUILD_CACHE[key]

    global LAST_RUN
    LAST_RUN = (nc, in_maps)

    results = run_bass_kernel_spmd(nc, in_maps, core_ids=list(range(NCORES))).results

    out = np.zeros((N, C, L + M), np.float32)
    out[:, :, :L] = x
    for q in range(NCORES):
        yq = (
            np.asarray(results[q]["yout"])
            .astype(np.float32)
            .reshape(128, S, 2, 32)
            .transpose(1, 3, 2, 0)
            .reshape(S, 32, 256)
        )
        for s in range(S):
            i = node_of_coreslot[q, s]
            if i >= 0:
                out[:, :, L + i] = yq[s]
    return out

